# revision 1
# baseline (speedup 1.0000x reference)
"""Trainium2 Bass kernel for nn_LocalSelfAttention (fused attention block).

Reference (B=2, S=2048, DM=1024, H=16, D=64):
  qkv = x @ Wqkv + bqkv -> split heads -> RoPE(q,k) -> softmax(q k^T/8) v
  -> concat heads @ Wo + bo -> residual + LayerNorm(gamma,beta)

Sharding (8 cores, no collectives): core c = (batch c//4, query rows
512*(c%4)..+512).  K/V projections for a batch are computed redundantly by
its 4 cores; attention/out-proj/LN are exact and row-local.  Host gather is
pure concatenation.

Device layouts: x^T on partitions (host-transposed, bf16); Wq/Wk columns
permuted per head to [evens, odds] so RoPE pairs sit 32 partitions apart;
scores computed transposed (S^T = K Q^T) so exp(P^T) feeds the V-matmul
directly; softmax row-sums via an appended ones-column on V (M=65); the
1/rowsum broadcast via a PE outer product.  All matmuls bf16 with fp32 PSUM
accumulation.
"""
import numpy as np
import ml_dtypes

import concourse.bass as bass
import concourse.mybir as mybir
import concourse.tile as tile
from concourse.bass_utils import run_bass_kernel_spmd

BF16 = ml_dtypes.bfloat16
bf16 = mybir.dt.bfloat16
f32 = mybir.dt.float32
AF = mybir.ActivationFunctionType
ALU = mybir.AluOpType
AX = mybir.AxisListType

B, S, DM = 2, 2048, 1024
H, D = 16, 64
NC = 8
ROWS = S * B // NC          # 512 query rows per core
SB = S


# ---- TileContext tail-drain patch: this walrus rejects >1 sync wait on
# CTRL-class instructions; split the global-clock waits onto SP nops.
def _patched_drain_and_barrier(self, tick_clock, wait_clock):
    nc = self.nc
    drain_inst = nc.sync.drain()
    wait_clock.add_sem_waits(
        drain_inst.ins, tile.ScopedClock({None: tick_clock.global_clock})
    )
    si = drain_inst.ins.sync_info
    waits = list(si.on_wait) if si and si.on_wait else []
    if len(waits) > 1:
        si.on_wait = waits[:1]
        for w in waits[1:]:
            nop = nc.sync.nop()
            nop.ins.sync_info = mybir.SyncInfo(on_wait=[w], on_update=[])
    nc.all_engine_barrier()
    assert self.sems is not None
    popped = nc._tile_sem_poison_stack.pop()
    assert popped is self._sem_poison
    # EVENT_SEMAPHORE_RANGE_CLEAR InstISA trips "ISA wrong length" in this
    # walrus; decrement each sem by its final (compile-time-known) value
    # instead so re-execution of the loaded NEFF starts from zero.
    # (sem decrement via EventSemaphore immediates overflows this walrus's
    # encoding; rely on NRT/PJRT resetting sem state between executions --
    # verified by the second-call check in test.py)
    nc.all_engine_barrier()


tile.TileContext._drain_and_barrier = _patched_drain_and_barrier

_CTRL_CLASSES = ("InstNoOp", "InstDrain", "InstEventSemaphore")


def _split_excess_waits(nc, maxw_compute=1):
    """Walrus (this version) caps sync waits per instruction (1 for
    CTRL-class, ~2 for compute).  Hoist excess waits onto same-engine NoOps
    inserted immediately before the offending instruction."""
    import copy
    proto = nc.sync.nop().ins  # prototype NoOp (appended to current bb; harmless)
    proto_si = proto.sync_info
    if proto_si and proto_si.on_wait:
        proto.sync_info = mybir.SyncInfo(on_wait=[], on_update=[])
    nsplit = 0
    for f in nc.m.functions:
        for b in f.blocks:
            insts = list(b.instructions)
            out = []
            changed = False
            for inst in insts:
                cls = type(inst).__name__
                maxw = 1 if cls in _CTRL_CLASSES else maxw_compute
                si = inst.sync_info
                waits = list(si.on_wait) if si and si.on_wait else []
                if len(waits) > maxw:
                    keep = waits[:maxw]
                    extra = waits[maxw:]
                    si.on_wait = keep
                    for i, w in enumerate(extra):
                        nop = copy.deepcopy(proto)
                        nop.name = f"{inst.name}-wsplit{i}"
                        nop.engine = inst.engine
                        nop.sync_info = mybir.SyncInfo(on_wait=[w],
                                                       on_update=[])
                        out.append(nop)
                        nsplit += 1
                    changed = True
                out.append(inst)
            if changed:
                try:
                    b.instructions = out
                except Exception:
                    b.set_instructions(out)
    return nsplit


def _build_program():
    nc = bass.Bass("TRN2", target_bir_lowering=False, debug=False,
                   num_devices=NC)

    def din(name, shape, dt):
        return nc.dram_tensor(name, list(shape), dt, kind="ExternalInput").ap()

    xT = din("xT", (DM, SB), bf16)
    xTq = din("xTq", (DM, ROWS), bf16)
    xr = din("xr", (ROWS, DM), f32)
    wq = din("wq", (DM, DM), bf16)
    wk = din("wk", (DM, DM), bf16)
    wv = din("wv", (DM, DM), bf16)
    wo = din("wo", (DM, DM), bf16)
    cc = din("cc", (128, SB), bf16)
    ss = din("ss", (128, SB), bf16)
    ccr = din("ccr", (128, ROWS), bf16)
    ssr = din("ssr", (128, ROWS), bf16)
    bqp = din("bqp", (128, 8), f32)
    bkp = din("bkp", (128, 8), f32)
    bvn = din("bvn", (128, 8), f32)
    gbc = din("gbc", (128, DM), f32)
    bbc = din("bbc", (128, DM), f32)
    obc = din("obc", (128, DM), f32)
    out = nc.dram_tensor("out", [ROWS, DM], f32, kind="ExternalOutput").ap()

    with tile.TileContext(nc) as tc:
        with tc.tile_pool(name="res", bufs=1) as res, \
             tc.tile_pool(name="tmp", bufs=3) as tmp, \
             tc.tile_pool(name="ppool", bufs=4) as ppool:

            xt_sb = [res.tile([128, SB], bf16, name=f"xt{k}", tag=f"xt{k}") for k in range(8)]
            xq_sb = [res.tile([128, ROWS], bf16, name=f"xq{k}", tag=f"xq{k}") for k in range(8)]
            kT = [res.tile([128, SB], bf16, name=f"kT{t}", tag=f"kT{t}") for t in range(8)]
            qT = [res.tile([128, ROWS], bf16, name=f"qT{t}", tag=f"qT{t}") for t in range(8)]
            vt = [res.tile([128, H * (D + 1)], bf16, name=f"vt{m}", tag=f"vt{m}")
                  for m in range(16)]
            aT = [res.tile([128, ROWS], bf16, name=f"aT{t}", tag=f"aT{t}") for t in range(8)]
            cc_sb = res.tile([128, SB], bf16, tag="cc")
            ss_sb = res.tile([128, SB], bf16, tag="ss")
            ccr_sb = res.tile([128, ROWS], bf16, tag="ccr")
            ssr_sb = res.tile([128, ROWS], bf16, tag="ssr")
            bq_sb = res.tile([128, 8], f32, tag="bq")
            bk_sb = res.tile([128, 8], f32, tag="bk")
            bv_sb = res.tile([128, 8], f32, tag="bv")
            ones_sb = res.tile([1, 64], bf16, tag="ones1")
            eps_sb = res.tile([128, 1], f32, tag="eps")

            for k in range(8):
                nc.sync.dma_start(xt_sb[k][:], xT[k * 128:(k + 1) * 128, :])
                nc.sync.dma_start(xq_sb[k][:], xTq[k * 128:(k + 1) * 128, :])
            nc.sync.dma_start(cc_sb[:], cc[:])
            nc.sync.dma_start(ss_sb[:], ss[:])
            nc.sync.dma_start(ccr_sb[:], ccr[:])
            nc.sync.dma_start(ssr_sb[:], ssr[:])
            nc.sync.dma_start(bq_sb[:], bqp[:])
            nc.sync.dma_start(bk_sb[:], bkp[:])
            nc.sync.dma_start(bv_sb[:], bvn[:])
            nc.vector.memset(ones_sb[:], 1.0)
            nc.vector.memset(eps_sb[:], 1e-5)

            def rope(dst, src, cct, sst, n0, nn):
                # dst[:, n0:n0+nn] = src*CC + swap32(src)*SS
                # (cross-partition 2-input DVE ops are illegal -> copy first)
                t1 = tmp.tile([128, nn], bf16, tag="ropet1")
                t2 = tmp.tile([128, nn], bf16, tag="ropet2")
                for a, b_ in ((0, 32), (32, 0), (64, 96), (96, 64)):
                    nc.vector.tensor_copy(t2[a:a + 32, :], src[b_:b_ + 32, :])
                nc.vector.tensor_tensor(out=t1[:], in0=src[:],
                                        in1=cct[:, n0:n0 + nn], op=ALU.mult)
                nc.vector.tensor_tensor(out=t2[:], in0=t2[:],
                                        in1=sst[:, n0:n0 + nn], op=ALU.mult)
                nc.vector.tensor_tensor(out=dst[:, n0:n0 + nn], in0=t1[:],
                                        in1=t2[:], op=ALU.add)

            # ---- projections ----
            with tc.tile_pool(name="wts", bufs=1) as wts, \
                 tc.tile_pool(name="psP", bufs=3, space="PSUM") as psP:
                wk_sb = [wts.tile([128, DM], bf16, name=f"wk{k}", tag=f"wk{k}")
                         for k in range(8)]
                wv_sb = [wts.tile([128, DM], bf16, name=f"wv{k}", tag=f"wv{k}")
                         for k in range(8)]
                for k in range(8):
                    nc.sync.dma_start(wk_sb[k][:], wk[k * 128:(k + 1) * 128, :])
                    nc.sync.dma_start(wv_sb[k][:], wv[k * 128:(k + 1) * 128, :])

                # K^T projection + RoPE
                for n in range(4):
                    n0 = n * 512
                    for t in range(8):
                        ps = psP.tile([128, 512], f32, tag="proj")
                        for kd in range(8):
                            nc.tensor.matmul(
                                ps[:], wk_sb[kd][:, t * 128:(t + 1) * 128],
                                xt_sb[kd][:, n0:n0 + 512],
                                start=(kd == 0), stop=(kd == 7))
                        kt_raw = tmp.tile([128, 512], bf16, tag="evac")
                        nc.scalar.activation(kt_raw[:], ps[:], AF.Identity,
                                             bias=bk_sb[:, t:t + 1])
                        rope(kT[t], kt_raw, cc_sb, ss_sb, n0, 512)

                # V projection (natural layout, 65-stride head slots)
                for m in range(16):
                    m0 = m * 128
                    for ncol in range(2):
                        c0 = ncol * 512
                        ps = psP.tile([128, 512], f32, tag="proj")
                        for kd in range(8):
                            nc.tensor.matmul(
                                ps[:], xt_sb[kd][:, m0:m0 + 128],
                                wv_sb[kd][:, c0:c0 + 512],
                                start=(kd == 0), stop=(kd == 7))
                        dst = vt[m][:, ncol * 8 * 65:(ncol + 1) * 8 * 65]
                        dstv = dst.rearrange("p (h e) -> p h e", e=65)[:, :, 0:64]
                        srcv = ps[:].rearrange("p (h e) -> p h e", e=64)
                        nc.scalar.activation(dstv, srcv, AF.Identity)
                    onev = vt[m][:, :].rearrange("p (h e) -> p h e",
                                                 e=65)[:, :, 64:65]
                    nc.vector.memset(onev, 1.0)

                # Q^T projection + RoPE (wq reuses wk slots)
                wq_sb = [wts.tile([128, DM], bf16, name=f"wq{k}", tag=f"wk{k}")
                         for k in range(8)]
                for k in range(8):
                    nc.sync.dma_start(wq_sb[k][:], wq[k * 128:(k + 1) * 128, :])
                for t in range(8):
                    ps = psP.tile([128, 512], f32, tag="proj")
                    for kd in range(8):
                        nc.tensor.matmul(
                            ps[:], wq_sb[kd][:, t * 128:(t + 1) * 128],
                            xq_sb[kd][:], start=(kd == 0), stop=(kd == 7))
                    q_raw = tmp.tile([128, ROWS], bf16, tag="evac")
                    nc.scalar.activation(q_raw[:], ps[:], AF.Identity,
                                         bias=bq_sb[:, t:t + 1])
                    rope(qT[t], q_raw, ccr_sb, ssr_sb, 0, ROWS)

            # ---- attention ----
            with tc.tile_pool(name="psA", bufs=3, space="PSUM") as psA, \
                 tc.tile_pool(name="psO", bufs=2, space="PSUM") as psO, \
                 tc.tile_pool(name="psB", bufs=1, space="PSUM") as psB:
                for h in range(H):
                    t, po = h // 2, 64 * (h % 2)
                    oacc = psO.tile([65, 512], f32, tag="oacc")
                    for kc in range(16):
                        k0 = kc * 128
                        sps = psA.tile([128, 512], f32, tag="sco")
                        nc.tensor.matmul(sps[:],
                                         kT[t][po:po + 64, k0:k0 + 128],
                                         qT[t][po:po + 64, :],
                                         start=True, stop=True)
                        pT = ppool.tile([128, 512], bf16, tag="pT")
                        nc.scalar.activation(pT[:], sps[:], AF.Exp,
                                             scale=0.125)
                        nc.tensor.matmul(oacc[:],
                                         vt[kc][:, h * 65:h * 65 + 65],
                                         pT[:], start=(kc == 0),
                                         stop=(kc == 15))
                    recip = tmp.tile([1, 512], bf16, tag="recip")
                    with nc.allow_low_precision(reason="softmax 1/rowsum in bf16"):
                        nc.vector.reciprocal(recip[:], oacc[64:65, :])
                    bc = psB.tile([64, 512], f32, tag="bc")
                    nc.tensor.matmul(bc[:], ones_sb[:], recip[:],
                                     start=True, stop=True)
                    bc_sb = tmp.tile([64, 512], bf16, tag="bcs")
                    nc.scalar.activation(bc_sb[:], bc[:], AF.Identity)
                    nc.vector.tensor_tensor(out=aT[t][po:po + 64, :],
                                            in0=oacc[0:64, :], in1=bc_sb[:],
                                            op=ALU.mult)
                    nc.vector.tensor_scalar(
                        out=aT[t][po:po + 64, :], in0=aT[t][po:po + 64, :],
                        scalar1=bv_sb[po:po + 64, h // 2:h // 2 + 1],
                        scalar2=None, op0=ALU.add)

            # ---- out-proj + residual + LayerNorm ----
            with tc.tile_pool(name="wop", bufs=1) as wop, \
                 tc.tile_pool(name="fin", bufs=1) as fin, \
                 tc.tile_pool(name="psF", bufs=2, space="PSUM") as psF:
                wo_sb = [wop.tile([128, DM], bf16, name=f"wo{k}", tag=f"wo{k}")
                         for k in range(8)]
                for k in range(8):
                    nc.sync.dma_start(wo_sb[k][:], wo[k * 128:(k + 1) * 128, :])
                g_sb = wop.tile([128, DM], f32, tag="g")
                b_sb = wop.tile([128, DM], f32, tag="b")
                o_sb = wop.tile([128, DM], f32, tag="o")
                nc.sync.dma_start(g_sb[:], gbc[:])
                nc.sync.dma_start(b_sb[:], bbc[:])
                nc.sync.dma_start(o_sb[:], obc[:])

                for mr in range(4):
                    rr = mr * 128
                    xb = fin.tile([128, DM], f32, tag="xb")
                    nc.sync.dma_start(xb[:], xr[rr:rr + 128, :])
                    hrow = fin.tile([128, DM], f32, tag="hrow")
                    for ncol in range(2):
                        c0 = ncol * 512
                        ps = psF.tile([128, 512], f32, tag="fin")
                        for kd in range(8):
                            nc.tensor.matmul(
                                ps[:], aT[kd][:, rr:rr + 128],
                                wo_sb[kd][:, c0:c0 + 512],
                                start=(kd == 0), stop=(kd == 7))
                        nc.vector.tensor_tensor(
                            out=hrow[:, c0:c0 + 512], in0=ps[:],
                            in1=xb[:, c0:c0 + 512], op=ALU.add)
                    nc.vector.tensor_tensor(out=hrow[:], in0=hrow[:],
                                            in1=o_sb[:], op=ALU.add)
                    ssum = fin.tile([128, 1], f32, tag="ssum")
                    nc.vector.reduce_sum(out=ssum[:], in_=hrow[:], axis=AX.X)
                    mu = fin.tile([128, 1], f32, tag="mu")
                    nc.vector.tensor_scalar(out=mu[:], in0=ssum[:],
                                            scalar1=1.0 / DM, scalar2=None,
                                            op0=ALU.mult)
                    d = fin.tile([128, DM], f32, tag="d")
                    nc.vector.tensor_scalar(out=d[:], in0=hrow[:],
                                            scalar1=mu[:], scalar2=None,
                                            op0=ALU.subtract)
                    y = fin.tile([128, DM], f32, tag="y")
                    vs = fin.tile([128, 1], f32, tag="vs")
                    nc.vector.tensor_tensor(out=y[:], in0=d[:], in1=d[:],
                                            op=ALU.mult)
                    nc.vector.reduce_sum(out=vs[:], in_=y[:], axis=AX.X)
                    st = fin.tile([128, 1], f32, tag="st")
                    nc.scalar.activation(st[:], vs[:], AF.Sqrt,
                                         bias=eps_sb[:], scale=1.0 / DM)
                    rstd = fin.tile([128, 1], f32, tag="rstd")
                    nc.vector.reciprocal(rstd[:], st[:])
                    nc.vector.tensor_scalar(out=y[:], in0=d[:],
                                            scalar1=rstd[:], scalar2=None,
                                            op0=ALU.mult)
                    nc.vector.tensor_tensor(out=y[:], in0=y[:], in1=g_sb[:],
                                            op=ALU.mult)
                    nc.vector.tensor_tensor(out=y[:], in0=y[:], in1=b_sb[:],
                                            op=ALU.add)
                    nc.sync.dma_start(out[rr:rr + 128, :], y[:])

    _split_excess_waits(nc)
    return nc


_NC_CACHE = None


def _perm():
    p = np.zeros(DM, np.int64)
    for h in range(H):
        p[h * D:h * D + 32] = h * D + np.arange(0, D, 2)
        p[h * D + 32:(h + 1) * D] = h * D + np.arange(1, D, 2)
    return p


def kernel(x, Wqkv, bqkv, Wo, bo, gamma, beta):
    global _NC_CACHE
    x = np.asarray(x, np.float32)
    Wqkv = np.asarray(Wqkv, np.float32)
    bqkv = np.asarray(bqkv, np.float32)
    Wo = np.asarray(Wo, np.float32)
    bo = np.asarray(bo, np.float32)
    gamma = np.asarray(gamma, np.float32)
    beta = np.asarray(beta, np.float32)

    perm = _perm()
    Wq = np.ascontiguousarray(Wqkv[:, 0:DM][:, perm]).astype(BF16)
    Wk = np.ascontiguousarray(Wqkv[:, DM:2 * DM][:, perm]).astype(BF16)
    Wv = np.ascontiguousarray(Wqkv[:, 2 * DM:3 * DM]).astype(BF16)
    Wob = Wo.astype(BF16)
    bq = bqkv[0:DM][perm]
    bk = bqkv[DM:2 * DM][perm]
    bv = bqkv[2 * DM:3 * DM]

    inv = 1.0 / (10000.0 ** (np.arange(0, D, 2, dtype=np.float64) / D))
    pos = np.arange(S, dtype=np.float64)
    fr = pos[None, :] * inv[:, None]                    # [32, S]
    c32, s32 = np.cos(fr), np.sin(fr)
    CC = np.concatenate([c32, c32, c32, c32], 0).astype(BF16)   # [128, S]
    SS = np.concatenate([-s32, s32, -s32, s32], 0).astype(BF16)

    def colmajor(v):
        return np.ascontiguousarray(v.reshape(8, 128).T).astype(np.float32)

    gB = np.ascontiguousarray(np.broadcast_to(gamma, (128, DM))).astype(np.float32)
    bB = np.ascontiguousarray(np.broadcast_to(beta, (128, DM))).astype(np.float32)
    oB = np.ascontiguousarray(np.broadcast_to(bo, (128, DM))).astype(np.float32)

    if _NC_CACHE is None:
        _NC_CACHE = _build_program()
    nc = _NC_CACHE

    in_maps = []
    for c in range(NC):
        b, r = c // 4, c % 4
        xTb = np.ascontiguousarray(x[b].T).astype(BF16)
        rr = r * ROWS
        in_maps.append({
            "xT": xTb,
            "xTq": np.ascontiguousarray(xTb[:, rr:rr + ROWS]),
            "xr": np.ascontiguousarray(x[b, rr:rr + ROWS, :]),
            "wq": Wq, "wk": Wk, "wv": Wv, "wo": Wob,
            "cc": CC, "ss": SS,
            "ccr": np.ascontiguousarray(CC[:, rr:rr + ROWS]),
            "ssr": np.ascontiguousarray(SS[:, rr:rr + ROWS]),
            "bqp": colmajor(bq), "bkp": colmajor(bk), "bvn": colmajor(bv),
            "gbc": gB, "bbc": bB, "obc": oB,
        })

    res = run_bass_kernel_spmd(nc, in_maps, core_ids=list(range(NC)))
    kernel._last_results = res
    full = np.empty((B, S, DM), np.float32)
    for c in range(NC):
        b, r = c // 4, c % 4
        full[b, r * ROWS:(r + 1) * ROWS, :] = res.results[c]["out"]
    return full

